# revision 40
# baseline (speedup 1.0000x reference)
"""Bahdanau-style attention kernel for Trainium2, 8 NeuronCores.

Reference computation (per batch b):
    score  = tanh(c @ W1 + W1_b + (h @ W2 + W2_b)[None, :])   # [T, U]
    logits = score @ V_w (+ V_b, cancels in softmax)          # [T, 1]
    attn   = softmax(logits over T)
    out    = sum_t attn[t] * c[t, :]                          # [D]

Sharding: pure data-parallel over batch B=64 across 8 cores (8 batches/core).
No collectives; host concatenates per-core outputs.

Host-side marshalling: c is cast to bf16 and shipped transposed [BL, D, T]
(the only layout the kernel needs). All FLOPs run on device.

Per-core dataflow ([u,t] orientation), per batch (T=2048 = 4 t-supers of 512):
  - 2MB of cT [d128, (k t)] per batch on the sync DMA queue, triple-buffered
    (pass-2 reads cts late; a second gpsimd queue makes things WORSE on HW).
  - main matmul on TensorE: psum_uc[u128, t512] += W1_chunk.T @ cT_chunk (bf16)
  - tanh on ScalarE with per-partition bias = (h@W2 + b)[u-chunk] -> score^T bf16
  - V-dot split PE/DVE (vdot_dve chunks on DVE): DVE chunks get per-partition
    V-scale (tensor_scalar, 4x mode) + bf16 add-tree; PE chunks use
    replicated-V lhsT matmuls. One shared PSUM accumulation group (vrep mms +
    one ones-matmul) does the u-partition sum AND broadcasts logits to all
    128 partitions. GpSimd is DMA-trigger only: its tensor ops cost 1.2-7.3us
    per [128,512] op on real HW (sw ucode dispatch), 3-18x the cost model.
  - exp on ScalarE -> w_row bf16, accum_out -> per-super softmax denominator
  - pass-2: fused multiply-reduce on DVE (scalar_tensor_tensor accum_out) per
    (d-chunk, super): ctx[d-chunk, slot] = sum_t cT_chunk * w_bcast
  - per-batch tail: reduce supers, transpose [128,4]->[4,128] on TensorE,
    divide by denominator on DVE, DMA out.

Measured: baseline 160.8us/iter; this version 114.7us/iter (repeat-slope, HW)
— 5us above the 109.2us bf16 matmul roofline. Keys: pso_bufs=2 (double-
buffered V-dot psum), single sync-queue cts load (split_load=False: a second
gpsimd DMA queue REGRESSES 114.7 -> 141), ct_bufs=3.
"""

import ml_dtypes
import numpy as np

import concourse.bass as bass
import concourse.tile as tile
from concourse import bacc, bass_isa, mybir
from concourse import bass_utils

B, T, D, U = 64, 2048, 512, 512
NCORES = 8
BL = B // NCORES  # 8 batches per core
KD = D // 128     # 4 contraction chunks
NST = T // 512    # 4 t-supers per batch
F32 = mybir.dt.float32
BF16 = mybir.dt.bfloat16
AF = mybir.ActivationFunctionType
ALU = mybir.AluOpType


def build_nc(n_batch=BL, repeat=1, stage=7, psp_bufs=5, pso_bufs=2, ct_bufs=3,
             score_bufs=12, work_bufs=6, sync_load=True, alloc_mode="stack",
             p2_pool_k=0, use_par=False, p2_wide=False, vdot_dve=3, p2_stt=True,
             split_load=False, mm_n1024=False):
    # stage: 1=loads 2=+main-mms 3=+tanh 4=+vdot 5=+exp 6=+pass2 7=full
    if mm_n1024:
        # [128,1024] f32 psum tiles span 2 banks; 3 bufs + pso 1 + ptp 1 = 8
        psp_bufs = min(psp_bufs, 3)
        pso_bufs = 1
        score_bufs = max(score_bufs, 20)
    nc = bacc.Bacc(None, target_bir_lowering=False)

    ct_ext = nc.declare_dram_parameter("ct", [BL, D, T], BF16, isOutput=False)
    ht_ext = nc.declare_dram_parameter("hT", [128, KD * 16], BF16, isOutput=False)
    w1_ext = nc.declare_dram_parameter("W1_w", [D, U], BF16, isOutput=False)
    b1_ext = nc.declare_dram_parameter("W1_b", [U], F32, isOutput=False)
    w2_ext = nc.declare_dram_parameter("W2_w", [D, U], BF16, isOutput=False)
    b2_ext = nc.declare_dram_parameter("W2_b", [U], F32, isOutput=False)
    v_ext = nc.declare_dram_parameter("V_w", [128, KD], F32, isOutput=False)
    ones_ext = nc.declare_dram_parameter("ones", [128, 128], F32, isOutput=False)
    eye_ext = nc.declare_dram_parameter("eye", [128, 128], F32, isOutput=False)
    out_ext = nc.declare_dram_parameter("out", [BL, D], F32, isOutput=True)

    with tile.TileContext(nc, pool_alloc_mode=alloc_mode) as tc:
        with (
            tc.tile_pool(name="const", bufs=1) as constp,
            tc.tile_pool(name="ct", bufs=ct_bufs) as ctp,
            tc.tile_pool(name="work", bufs=work_bufs) as workp,
            tc.tile_pool(name="score", bufs=score_bufs) as scorep,
            tc.tile_pool(name="sv", bufs=8) as svp,
        ):
            # ---------------- setup (one-time) ----------------
            with tc.tile_pool(name="spsum", bufs=1, space="PSUM") as sps:
                ones_f = constp.tile([128, 128], F32)
                nc.gpsimd.dma_start(ones_f[:], ones_ext[:, :])
                ones_bf = constp.tile([128, 128], BF16)
                nc.scalar.activation(ones_bf[:], ones_f[:], AF.Copy)
                eye_f = constp.tile([128, 128], F32)
                nc.gpsimd.dma_start(eye_f[:], eye_ext[:, :])

                # W1 chunks [d128, (k u)] bf16 (pre-converted on host):
                # lhsT slice [d, u-chunk]
                w1_bf = constp.tile([128, KD * U], BF16)
                nc.gpsimd.dma_start(
                    w1_bf[:].rearrange("p (k u) -> p k u", k=KD),
                    w1_ext.rearrange("(k p) u -> p k u", p=128),
                )
                w2_bf = constp.tile([128, KD * U], BF16)
                nc.scalar.dma_start(
                    w2_bf[:].rearrange("p (k u) -> p k u", k=KD),
                    w2_ext.rearrange("(k p) u -> p k u", p=128),
                )

                # hT [d128, (k 16)] bf16 pre-transposed on host
                hT_bf = constp.tile([128, KD * 16], BF16)
                nc.sync.dma_start(hT_bf[:], ht_ext[:, :])

                b1_f = constp.tile([1, U], F32)
                nc.gpsimd.dma_start(b1_f[:], b1_ext[None, :])
                b2_f = constp.tile([1, U], F32)
                nc.gpsimd.dma_start(b2_f[:], b2_ext[None, :])
                b12_f = constp.tile([1, U], F32)
                nc.vector.tensor_add(b12_f[:], b1_f[:], b2_f[:])
                b12_bf = constp.tile([1, U], BF16)
                nc.scalar.activation(b12_bf[:], b12_f[:], AF.Copy)

                # hb[b, u] = h[b] @ W2 + W1_b + W2_b   (rows 0:BL valid)
                ps_hb = sps.tile([16, U], F32)
                for k in range(KD):
                    nc.tensor.matmul(
                        ps_hb[:],
                        lhsT=hT_bf[:, 16 * k : 16 * (k + 1)],
                        rhs=w2_bf[:, U * k : U * (k + 1)],
                        start=(k == 0),
                        stop=False,
                    )
                nc.tensor.matmul(
                    ps_hb[:], lhsT=ones_bf[0:1, 0:16], rhs=b12_bf[:],
                    start=False, stop=True,
                )
                hbr_f = constp.tile([16, U], F32)
                nc.scalar.activation(hbr_f[:], ps_hb[:], AF.Copy)
                # transpose to hbT [u128, (k 16)] f32 (tanh bias columns)
                ps_hbt = sps.tile([128, KD * 16], F32)
                for k in range(KD):
                    nc.tensor.transpose(
                        ps_hbt[:, 16 * k : 16 * (k + 1)],
                        hbr_f[0:16, 128 * k : 128 * (k + 1)],
                        eye_f[0:16, 0:16],
                    )
                hbT_f = constp.tile([128, KD * 16], F32)
                nc.scalar.activation(hbT_f[:], ps_hbt[:], AF.Copy)

                # V columns: vcol_f[u128, k] = V[u-chunk k] (host pre-arranged)
                vcol_f = constp.tile([128, KD], F32)
                nc.scalar.dma_start(vcol_f[:], v_ext[:, :])
                # V replicated: vrep_k [u128, 128] bf16, every column = V[chunk k]
                vrep_bf = constp.tile([128, KD * 128], BF16)
                for k in range(KD):
                    nc.vector.tensor_scalar(
                        out=vrep_bf[:, 128 * k : 128 * (k + 1)],
                        in0=ones_f[:, :],
                        scalar1=0.0,
                        scalar2=vcol_f[:, k : k + 1],
                        op0=ALU.mult,
                        op1=ALU.add,
                    )

            # ---------------- main loop ----------------
            with (
                tc.tile_pool(name="psum_s", bufs=psp_bufs, space="PSUM") as psp,
                tc.tile_pool(name="psum_o", bufs=pso_bufs, space="PSUM") as psop,
                tc.tile_pool(name="psum_t", bufs=1, space="PSUM") as ptp,
            ):
                for rep in range(repeat):
                  ctxn_all = constp.tile([128, n_batch * KD], F32, tag=f"ctxnall{rep}")
                  for b in range(n_batch):
                    cts = ctp.tile([128, KD * T], BF16)
                    if split_load:
                        # two DMA queues (sync + gpsimd) halve per-batch load
                        # latency and double aggregate load bandwidth
                        ctv = cts[:].rearrange("p (k t) -> p k t", k=KD)
                        csrc = ct_ext[b].rearrange("(k p) t -> p k t", p=128)
                        nc.sync.dma_start(ctv[:, 0:2], csrc[:, 0:2])
                        nc.gpsimd.dma_start(ctv[:, 2:4], csrc[:, 2:4])
                    else:
                        load_eng = nc.sync if sync_load else nc.gpsimd
                        load_eng.dma_start(
                            cts[:].rearrange("p (k t) -> p k t", k=KD),
                            ct_ext[b].rearrange("(k p) t -> p k t", p=128),
                        )
                    if stage < 2:
                        continue
                    if not p2_wide:
                        ctx_all = workp.tile([128, KD * NST], F32, tag="ctxall")
                    else:
                        wb_all = workp.tile([128, T], BF16, tag="wball")
                    s_all = workp.tile([128, NST], F32, tag="sall")
                    scs_all = [[None] * KD for _ in range(NST)]
                    if mm_n1024:
                        # N=1024 moving operand: half the matmul count and
                        # half the per-matmul weight loads. psum tiles span
                        # 2 banks; tanh consumes 512-wide halves.
                        for stp in range(NST // 2):
                            t0 = 1024 * stp
                            for uc in range(KD):
                                ps = psp.tile([128, 1024], F32, tag="ps")
                                for k in range(KD):
                                    nc.tensor.matmul(
                                        ps[:],
                                        lhsT=w1_bf[
                                            :, U * k + 128 * uc : U * k + 128 * (uc + 1)
                                        ],
                                        rhs=cts[:, T * k + t0 : T * k + t0 + 1024],
                                        start=(k == 0),
                                        stop=(k == KD - 1),
                                    )
                                if stage < 3:
                                    continue
                                for half in range(2):
                                    score = scorep.tile([128, 512], BF16, tag="score")
                                    nc.scalar.activation(
                                        score[:],
                                        ps[:, 512 * half : 512 * (half + 1)],
                                        AF.Tanh,
                                        bias=hbT_f[:, 16 * uc + b : 16 * uc + b + 1],
                                    )
                                    scs_all[2 * stp + half][uc] = score
                    for st in range(NST):
                        t0 = 512 * st
                        if not mm_n1024:
                            pss = []
                            for uc in range(KD):
                                ps = psp.tile([128, 512], F32, tag="ps")
                                pss.append(ps)
                                for k in range(KD):
                                    nc.tensor.matmul(
                                        ps[:],
                                        lhsT=w1_bf[:, U * k + 128 * uc : U * k + 128 * (uc + 1)],
                                        rhs=cts[:, T * k + t0 : T * k + t0 + 512],
                                        start=(k == 0),
                                        stop=(k == KD - 1),
                                    )
                        if stage < 3:
                            continue
                        if mm_n1024:
                            scs = scs_all[st]
                        else:
                            scs = []
                            for uc in range(KD):
                                score = scorep.tile([128, 512], BF16, tag="score")
                                scs.append(score)
                                nc.scalar.activation(
                                    score[:], pss[uc][:], AF.Tanh,
                                    bias=hbT_f[:, 16 * uc + b : 16 * uc + b + 1],
                                )
                        if stage < 4:
                            continue
                        # V-dot, split PE/DVE (vdot_dve chunks on DVE):
                        # chunks [0, nv) via replicated-V matmuls (V applied
                        # inside the matmul); chunks [nv, KD) V-scaled on DVE
                        # (tensor_scalar 4x mode) + add-tree, then one
                        # ones-matmul sums partitions + broadcasts. All
                        # matmuls share one PSUM accumulation group:
                        # lg[p,t] = logits[t] for every p.
                        nv = KD - vdot_dve
                        if vdot_dve > 0:
                            scaled = []
                            for j in range(vdot_dve):
                                uc = nv + j
                                aj = svp.tile([128, 512], BF16, tag=f"a{j}")
                                nc.vector.tensor_scalar_mul(
                                    aj[:], scs[uc][:], vcol_f[:, uc : uc + 1]
                                )
                                scaled.append(aj)
                            lvl = 0
                            while len(scaled) > 1:
                                nxt = []
                                for i2 in range(0, len(scaled) - 1, 2):
                                    s = svp.tile(
                                        [128, 512], BF16, tag=f"s{lvl}_{i2}"
                                    )
                                    nc.vector.tensor_add(
                                        s[:], scaled[i2][:], scaled[i2 + 1][:]
                                    )
                                    nxt.append(s)
                                if len(scaled) % 2:
                                    nxt.append(scaled[-1])
                                scaled = nxt
                                lvl += 1
                        lg = psop.tile([128, 512], F32, tag="pso")
                        nmm = nv + (1 if vdot_dve else 0)
                        im = 0
                        for uc in range(nv):
                            nc.tensor.matmul(
                                lg[:],
                                lhsT=vrep_bf[:, 128 * uc : 128 * (uc + 1)],
                                rhs=scs[uc][:],
                                start=(im == 0),
                                stop=(im == nmm - 1),
                            )
                            im += 1
                        if vdot_dve:
                            nc.tensor.matmul(
                                lg[:], lhsT=ones_bf[:, :], rhs=scaled[0][:],
                                start=(im == 0), stop=(im == nmm - 1),
                            )
                        if stage < 5:
                            continue
                        wtile = wb_all if p2_wide else workp.tile(
                            [128, 512], BF16, tag="wb"
                        )
                        wslice = wtile[:, t0 : t0 + 512] if p2_wide else wtile[:]
                        nc.scalar.activation(
                            wslice, lg[:], AF.Exp,
                            accum_out=s_all[:, st : st + 1],
                        )
                        if stage < 6:
                            continue
                        if not p2_wide:
                            for k in range(KD):
                                prod2 = workp.tile([128, 512], BF16, tag="prod2")
                                if k < p2_pool_k:
                                    # Pool multiply + DVE reduce
                                    nc.gpsimd.tensor_mul(
                                        prod2[:],
                                        cts[:, T * k + t0 : T * k + t0 + 512],
                                        wslice,
                                    )
                                    nc.vector.reduce_sum(
                                        ctx_all[:, NST * k + st : NST * k + st + 1],
                                        prod2[:],
                                        axis=mybir.AxisListType.X,
                                    )
                                elif p2_stt:
                                    # fused multiply-reduce on DVE
                                    nc.vector.scalar_tensor_tensor(
                                        out=prod2[:],
                                        in0=cts[:, T * k + t0 : T * k + t0 + 512],
                                        scalar=1.0,
                                        in1=wslice,
                                        op0=ALU.mult,
                                        op1=ALU.mult,
                                        accum_out=ctx_all[
                                            :, NST * k + st : NST * k + st + 1
                                        ],
                                    )
                                else:
                                    # DVE multiply + DVE reduce
                                    nc.vector.tensor_mul(
                                        prod2[:],
                                        cts[:, T * k + t0 : T * k + t0 + 512],
                                        wslice,
                                    )
                                    nc.vector.reduce_sum(
                                        ctx_all[:, NST * k + st : NST * k + st + 1],
                                        prod2[:],
                                        axis=mybir.AxisListType.X,
                                    )
                    if p2_wide and stage >= 6:
                        ctxs = workp.tile([128, KD], F32, tag="ctxs")
                        for k in range(KD):
                            prod2 = workp.tile([128, T], BF16, tag="prod2w")
                            nc.vector.scalar_tensor_tensor(
                                out=prod2[:],
                                in0=cts[:, T * k : T * (k + 1)],
                                scalar=1.0,
                                in1=wb_all[:],
                                op0=ALU.mult,
                                op1=ALU.mult,
                                accum_out=ctxs[:, k : k + 1],
                            )
                    if stage < 7:
                        continue
                    # ---- per-batch tail (DVE only; transpose batched at end) ----
                    stot = workp.tile([128, 1], F32, tag="stot")
                    nc.vector.reduce_sum(stot[:], s_all[:], axis=mybir.AxisListType.X)
                    invc = workp.tile([128, 1], F32, tag="invc")
                    nc.vector.reciprocal(invc[:], stot[:])
                    if not p2_wide:
                        ctxs = workp.tile([128, KD], F32, tag="ctxs")
                        for k in range(KD):
                            nc.vector.reduce_sum(
                                ctxs[:, k : k + 1],
                                ctx_all[:, NST * k : NST * (k + 1)],
                                axis=mybir.AxisListType.X,
                            )
                    nc.vector.tensor_scalar_mul(
                        ctxn_all[:, KD * b : KD * (b + 1)], ctxs[:], invc[:, 0:1]
                    )
                  if stage >= 7:
                    # ---- end-of-repeat tail: one transpose, one copy, one DMA ----
                    pst = ptp.tile([n_batch * KD, 128], F32, tag="pst")
                    nc.tensor.transpose(pst[:], ctxn_all[:], eye_f[:, :])
                    orows = workp.tile([n_batch * KD, 128], F32, tag="orows")
                    nc.scalar.activation(orows[:], pst[:], AF.Copy)
                    nc.gpsimd.dma_start(
                        out_ext.rearrange("b (k f) -> (b k) f", k=KD), orows[:]
                    )
    nc.compile()
    return nc


_NC_CACHE = None


def _get_nc():
    global _NC_CACHE
    if _NC_CACHE is None:
        _NC_CACHE = build_nc()
    return _NC_CACHE


def make_in_maps(c, h, W1_w, W1_b, W2_w, W2_b, V_w):
    c = np.asarray(c, np.float32)
    cb = c.astype(ml_dtypes.bfloat16)                    # [B, T, D] bf16
    ct = np.ascontiguousarray(cb.swapaxes(1, 2))         # [B, D, T] bf16
    shared = {
        "W1_w": np.ascontiguousarray(
            np.asarray(W1_w, np.float32).astype(ml_dtypes.bfloat16)
        ),
        "W1_b": np.ascontiguousarray(np.asarray(W1_b, np.float32)),
        "W2_w": np.ascontiguousarray(
            np.asarray(W2_w, np.float32).astype(ml_dtypes.bfloat16)
        ),
        "W2_b": np.ascontiguousarray(np.asarray(W2_b, np.float32)),
        # V columns: [p, k] = V[k*128 + p]
        "V_w": np.ascontiguousarray(
            np.asarray(V_w, np.float32).reshape(KD, 128).T
        ),
        "ones": np.ones((128, 128), np.float32),
        "eye": np.eye(128, dtype=np.float32),
    }
    h = np.asarray(h, np.float32)
    in_maps = []
    for i in range(NCORES):
        m = dict(shared)
        m["ct"] = ct[i * BL : (i + 1) * BL]
        # hT[p, k*16 + b] = h[b, k*128 + p], bf16
        hc = h[i * BL : (i + 1) * BL]                       # [BL, D]
        ht = np.zeros((128, KD * 16), np.float32)
        ht[:, :] = np.concatenate(
            [
                np.pad(hc[:, k * 128 : (k + 1) * 128].T, ((0, 0), (0, 16 - BL)))
                for k in range(KD)
            ],
            axis=1,
        )
        m["hT"] = np.ascontiguousarray(ht.astype(ml_dtypes.bfloat16))
        in_maps.append(m)
    return in_maps


def kernel(**inputs):
    in_maps = make_in_maps(
        inputs["c"], inputs["h"], inputs["W1_w"], inputs["W1_b"],
        inputs["W2_w"], inputs["W2_b"], inputs["V_w"],
    )
    nc = _get_nc()
    res = bass_utils.run_bass_kernel_spmd(nc, in_maps, core_ids=list(range(NCORES)))
    out = np.concatenate([np.asarray(r["out"]) for r in res.results], axis=0)
    return out.astype(np.float32)


if __name__ == "__main__":
    rng = np.random.default_rng(0)
    ins = {
        "c": rng.standard_normal((B, T, D), dtype=np.float32),
        "h": rng.standard_normal((B, D), dtype=np.float32),
        "W1_w": rng.standard_normal((D, U), dtype=np.float32) / np.sqrt(D),
        "W1_b": np.zeros((U,), np.float32),
        "W2_w": rng.standard_normal((D, U), dtype=np.float32) / np.sqrt(D),
        "W2_b": np.zeros((U,), np.float32),
        "V_w": rng.standard_normal((U, 1), dtype=np.float32) / np.sqrt(U),
        "V_b": np.zeros((1,), np.float32),
    }
    out = kernel(**ins)
    print("out", out.shape, out.dtype, np.abs(out).mean())


# revision 43
# speedup vs baseline: 1.1059x; 1.1059x over previous
"""Bahdanau-style attention kernel for Trainium2, 8 NeuronCores.

Reference computation (per batch b):
    score  = tanh(c @ W1 + W1_b + (h @ W2 + W2_b)[None, :])   # [T, U]
    logits = score @ V_w (+ V_b, cancels in softmax)          # [T, 1]
    attn   = softmax(logits over T)
    out    = sum_t attn[t] * c[t, :]                          # [D]

Sharding: pure data-parallel over batch B=64 across 8 cores (8 batches/core).
No collectives; host concatenates per-core outputs.

Host-side marshalling: c is cast to bf16 and shipped transposed [BL, D, T]
(the only layout the kernel needs). All FLOPs run on device.

Per-core dataflow ([u,t] orientation), per batch (T=2048 = 4 t-supers of 512):
  - 2MB of cT [d128, (k t)] per batch on the sync DMA queue, triple-buffered
    (pass-2 reads cts late; a second gpsimd queue makes things WORSE on HW).
  - main matmul on TensorE: psum_uc[u128, t512] += W1_chunk.T @ cT_chunk (bf16)
  - tanh on ScalarE with per-partition bias = (h@W2 + b)[u-chunk] -> score^T bf16
  - V-dot split PE/DVE (vdot_dve chunks on DVE): DVE chunks get per-partition
    V-scale (tensor_scalar, 4x mode) + bf16 add-tree; PE chunks use
    replicated-V lhsT matmuls. One shared PSUM accumulation group (vrep mms +
    one ones-matmul) does the u-partition sum AND broadcasts logits to all
    128 partitions. GpSimd is DMA-trigger only: its tensor ops cost 1.2-7.3us
    per [128,512] op on real HW (sw ucode dispatch), 3-18x the cost model.
  - exp on ScalarE -> w_row bf16, accum_out -> per-super softmax denominator
  - pass-2: fused multiply-reduce on DVE (scalar_tensor_tensor accum_out) per
    (d-chunk, super): ctx[d-chunk, slot] = sum_t cT_chunk * w_bcast
  - per-batch tail: reduce supers, transpose [128,4]->[4,128] on TensorE,
    divide by denominator on DVE, DMA out.

Measured: baseline 160.8us/iter; this version 114.7us/iter (repeat-slope, HW)
— 5us above the 109.2us bf16 matmul roofline. Keys: pso_bufs=2 (double-
buffered V-dot psum), single sync-queue cts load (split_load=False: a second
gpsimd DMA queue REGRESSES 114.7 -> 141), ct_bufs=3.
"""

import ml_dtypes
import numpy as np

import concourse.bass as bass
import concourse.tile as tile
from concourse import bacc, bass_isa, mybir
from concourse import bass_utils

B, T, D, U = 64, 2048, 512, 512
NCORES = 8
BL = B // NCORES  # 8 batches per core
KD = D // 128     # 4 contraction chunks
NST = T // 512    # 4 t-supers per batch
F32 = mybir.dt.float32
BF16 = mybir.dt.bfloat16
AF = mybir.ActivationFunctionType
ALU = mybir.AluOpType


def build_nc(n_batch=BL, repeat=1, stage=7, psp_bufs=5, pso_bufs=2, ct_bufs=4,
             score_bufs=12, work_bufs=6, sync_load=True, alloc_mode="stack",
             p2_pool_k=0, use_par=False, p2_wide=False, vdot_dve=3, p2_stt=True,
             split_load=False, mm_n1024=False, alt_load=False):
    # stage: 1=loads 2=+main-mms 3=+tanh 4=+vdot 5=+exp 6=+pass2 7=full
    if mm_n1024:
        # [128,1024] f32 psum tiles span 2 banks; 3 bufs + pso 1 + ptp 1 = 8
        psp_bufs = min(psp_bufs, 3)
        pso_bufs = 1
        score_bufs = max(score_bufs, 20)
    nc = bacc.Bacc(None, target_bir_lowering=False)

    ct_ext = nc.declare_dram_parameter("ct", [BL, D, T], BF16, isOutput=False)
    ht_ext = nc.declare_dram_parameter("hT", [128, KD * 16], BF16, isOutput=False)
    w1_ext = nc.declare_dram_parameter("W1_w", [D, U], BF16, isOutput=False)
    b1_ext = nc.declare_dram_parameter("W1_b", [U], F32, isOutput=False)
    w2_ext = nc.declare_dram_parameter("W2_w", [D, U], BF16, isOutput=False)
    b2_ext = nc.declare_dram_parameter("W2_b", [U], F32, isOutput=False)
    v_ext = nc.declare_dram_parameter("V_w", [128, KD], F32, isOutput=False)
    ones_ext = nc.declare_dram_parameter("ones", [128, 128], F32, isOutput=False)
    eye_ext = nc.declare_dram_parameter("eye", [128, 128], F32, isOutput=False)
    out_ext = nc.declare_dram_parameter("out", [BL, D], F32, isOutput=True)

    with tile.TileContext(nc, pool_alloc_mode=alloc_mode) as tc:
        with (
            tc.tile_pool(name="const", bufs=1) as constp,
            tc.tile_pool(name="ct", bufs=ct_bufs) as ctp,
            tc.tile_pool(name="work", bufs=work_bufs) as workp,
            tc.tile_pool(name="score", bufs=score_bufs) as scorep,
            tc.tile_pool(name="sv", bufs=8) as svp,
        ):
            # ---------------- setup (one-time) ----------------
            with tc.tile_pool(name="spsum", bufs=1, space="PSUM") as sps:
                ones_f = constp.tile([128, 128], F32)
                nc.gpsimd.dma_start(ones_f[:], ones_ext[:, :])
                ones_bf = constp.tile([128, 128], BF16)
                nc.scalar.activation(ones_bf[:], ones_f[:], AF.Copy)
                eye_f = constp.tile([128, 128], F32)
                nc.gpsimd.dma_start(eye_f[:], eye_ext[:, :])

                # W1 chunks [d128, (k u)] bf16 (pre-converted on host):
                # lhsT slice [d, u-chunk]
                w1_bf = constp.tile([128, KD * U], BF16)
                nc.gpsimd.dma_start(
                    w1_bf[:].rearrange("p (k u) -> p k u", k=KD),
                    w1_ext.rearrange("(k p) u -> p k u", p=128),
                )
                w2_bf = constp.tile([128, KD * U], BF16)
                nc.scalar.dma_start(
                    w2_bf[:].rearrange("p (k u) -> p k u", k=KD),
                    w2_ext.rearrange("(k p) u -> p k u", p=128),
                )

                # hT [d128, (k 16)] bf16 pre-transposed on host
                hT_bf = constp.tile([128, KD * 16], BF16)
                nc.sync.dma_start(hT_bf[:], ht_ext[:, :])

                b1_f = constp.tile([1, U], F32)
                nc.gpsimd.dma_start(b1_f[:], b1_ext[None, :])
                b2_f = constp.tile([1, U], F32)
                nc.gpsimd.dma_start(b2_f[:], b2_ext[None, :])
                b12_f = constp.tile([1, U], F32)
                nc.vector.tensor_add(b12_f[:], b1_f[:], b2_f[:])
                b12_bf = constp.tile([1, U], BF16)
                nc.scalar.activation(b12_bf[:], b12_f[:], AF.Copy)

                # hb[b, u] = h[b] @ W2 + W1_b + W2_b   (rows 0:BL valid)
                ps_hb = sps.tile([16, U], F32)
                for k in range(KD):
                    nc.tensor.matmul(
                        ps_hb[:],
                        lhsT=hT_bf[:, 16 * k : 16 * (k + 1)],
                        rhs=w2_bf[:, U * k : U * (k + 1)],
                        start=(k == 0),
                        stop=False,
                    )
                nc.tensor.matmul(
                    ps_hb[:], lhsT=ones_bf[0:1, 0:16], rhs=b12_bf[:],
                    start=False, stop=True,
                )
                hbr_f = constp.tile([16, U], F32)
                nc.scalar.activation(hbr_f[:], ps_hb[:], AF.Copy)
                # transpose to hbT [u128, (k 16)] f32 (tanh bias columns)
                ps_hbt = sps.tile([128, KD * 16], F32)
                for k in range(KD):
                    nc.tensor.transpose(
                        ps_hbt[:, 16 * k : 16 * (k + 1)],
                        hbr_f[0:16, 128 * k : 128 * (k + 1)],
                        eye_f[0:16, 0:16],
                    )
                hbT_f = constp.tile([128, KD * 16], F32)
                nc.scalar.activation(hbT_f[:], ps_hbt[:], AF.Copy)

                # V columns: vcol_f[u128, k] = V[u-chunk k] (host pre-arranged)
                vcol_f = constp.tile([128, KD], F32)
                nc.scalar.dma_start(vcol_f[:], v_ext[:, :])
                # V replicated: vrep_k [u128, 128] bf16, every column = V[chunk k]
                vrep_bf = constp.tile([128, KD * 128], BF16)
                for k in range(KD):
                    nc.vector.tensor_scalar(
                        out=vrep_bf[:, 128 * k : 128 * (k + 1)],
                        in0=ones_f[:, :],
                        scalar1=0.0,
                        scalar2=vcol_f[:, k : k + 1],
                        op0=ALU.mult,
                        op1=ALU.add,
                    )

            # ---------------- main loop ----------------
            with (
                tc.tile_pool(name="psum_s", bufs=psp_bufs, space="PSUM") as psp,
                tc.tile_pool(name="psum_o", bufs=pso_bufs, space="PSUM") as psop,
                tc.tile_pool(name="psum_t", bufs=1, space="PSUM") as ptp,
            ):
                for rep in range(repeat):
                  ctxn_all = constp.tile([128, n_batch * KD], F32, tag=f"ctxnall{rep}")
                  for b in range(n_batch):
                    cts = ctp.tile([128, KD * T], BF16)
                    if split_load:
                        # two DMA queues (sync + gpsimd) halve per-batch load
                        # latency and double aggregate load bandwidth
                        ctv = cts[:].rearrange("p (k t) -> p k t", k=KD)
                        csrc = ct_ext[b].rearrange("(k p) t -> p k t", p=128)
                        nc.sync.dma_start(ctv[:, 0:2], csrc[:, 0:2])
                        nc.gpsimd.dma_start(ctv[:, 2:4], csrc[:, 2:4])
                    else:
                        if alt_load:
                            # alternate whole-batch loads between two queues
                            load_eng = nc.sync if b % 2 == 0 else nc.gpsimd
                        else:
                            load_eng = nc.sync if sync_load else nc.gpsimd
                        load_eng.dma_start(
                            cts[:].rearrange("p (k t) -> p k t", k=KD),
                            ct_ext[b].rearrange("(k p) t -> p k t", p=128),
                        )
                    if stage < 2:
                        continue
                    if not p2_wide:
                        ctx_all = workp.tile([128, KD * NST], F32, tag="ctxall")
                    else:
                        wb_all = workp.tile([128, T], BF16, tag="wball")
                    s_all = workp.tile([128, NST], F32, tag="sall")
                    scs_all = [[None] * KD for _ in range(NST)]
                    if mm_n1024:
                        # N=1024 moving operand: half the matmul count and
                        # half the per-matmul weight loads. psum tiles span
                        # 2 banks; tanh consumes 512-wide halves.
                        for stp in range(NST // 2):
                            t0 = 1024 * stp
                            for uc in range(KD):
                                ps = psp.tile([128, 1024], F32, tag="ps")
                                for k in range(KD):
                                    nc.tensor.matmul(
                                        ps[:],
                                        lhsT=w1_bf[
                                            :, U * k + 128 * uc : U * k + 128 * (uc + 1)
                                        ],
                                        rhs=cts[:, T * k + t0 : T * k + t0 + 1024],
                                        start=(k == 0),
                                        stop=(k == KD - 1),
                                    )
                                if stage < 3:
                                    continue
                                for half in range(2):
                                    score = scorep.tile([128, 512], BF16, tag="score")
                                    nc.scalar.activation(
                                        score[:],
                                        ps[:, 512 * half : 512 * (half + 1)],
                                        AF.Tanh,
                                        bias=hbT_f[:, 16 * uc + b : 16 * uc + b + 1],
                                    )
                                    scs_all[2 * stp + half][uc] = score
                    for st in range(NST):
                        t0 = 512 * st
                        if not mm_n1024:
                            pss = []
                            for uc in range(KD):
                                ps = psp.tile([128, 512], F32, tag="ps")
                                pss.append(ps)
                                for k in range(KD):
                                    nc.tensor.matmul(
                                        ps[:],
                                        lhsT=w1_bf[:, U * k + 128 * uc : U * k + 128 * (uc + 1)],
                                        rhs=cts[:, T * k + t0 : T * k + t0 + 512],
                                        start=(k == 0),
                                        stop=(k == KD - 1),
                                    )
                        if stage < 3:
                            continue
                        if mm_n1024:
                            scs = scs_all[st]
                        else:
                            scs = []
                            for uc in range(KD):
                                score = scorep.tile([128, 512], BF16, tag="score")
                                scs.append(score)
                                nc.scalar.activation(
                                    score[:], pss[uc][:], AF.Tanh,
                                    bias=hbT_f[:, 16 * uc + b : 16 * uc + b + 1],
                                )
                        if stage < 4:
                            continue
                        # V-dot, split PE/DVE (vdot_dve chunks on DVE):
                        # chunks [0, nv) via replicated-V matmuls (V applied
                        # inside the matmul); chunks [nv, KD) V-scaled on DVE
                        # (tensor_scalar 4x mode) + add-tree, then one
                        # ones-matmul sums partitions + broadcasts. All
                        # matmuls share one PSUM accumulation group:
                        # lg[p,t] = logits[t] for every p.
                        nv = KD - vdot_dve
                        if vdot_dve > 0:
                            scaled = []
                            for j in range(vdot_dve):
                                uc = nv + j
                                aj = svp.tile([128, 512], BF16, tag=f"a{j}")
                                nc.vector.tensor_scalar_mul(
                                    aj[:], scs[uc][:], vcol_f[:, uc : uc + 1]
                                )
                                scaled.append(aj)
                            lvl = 0
                            while len(scaled) > 1:
                                nxt = []
                                for i2 in range(0, len(scaled) - 1, 2):
                                    s = svp.tile(
                                        [128, 512], BF16, tag=f"s{lvl}_{i2}"
                                    )
                                    nc.vector.tensor_add(
                                        s[:], scaled[i2][:], scaled[i2 + 1][:]
                                    )
                                    nxt.append(s)
                                if len(scaled) % 2:
                                    nxt.append(scaled[-1])
                                scaled = nxt
                                lvl += 1
                        lg = psop.tile([128, 512], F32, tag="pso")
                        nmm = nv + (1 if vdot_dve else 0)
                        im = 0
                        for uc in range(nv):
                            nc.tensor.matmul(
                                lg[:],
                                lhsT=vrep_bf[:, 128 * uc : 128 * (uc + 1)],
                                rhs=scs[uc][:],
                                start=(im == 0),
                                stop=(im == nmm - 1),
                            )
                            im += 1
                        if vdot_dve:
                            nc.tensor.matmul(
                                lg[:], lhsT=ones_bf[:, :], rhs=scaled[0][:],
                                start=(im == 0), stop=(im == nmm - 1),
                            )
                        if stage < 5:
                            continue
                        wtile = wb_all if p2_wide else workp.tile(
                            [128, 512], BF16, tag="wb"
                        )
                        wslice = wtile[:, t0 : t0 + 512] if p2_wide else wtile[:]
                        nc.scalar.activation(
                            wslice, lg[:], AF.Exp,
                            accum_out=s_all[:, st : st + 1],
                        )
                        if stage < 6:
                            continue
                        if not p2_wide:
                            for k in range(KD):
                                prod2 = workp.tile([128, 512], BF16, tag="prod2")
                                if k < p2_pool_k:
                                    # Pool multiply + DVE reduce
                                    nc.gpsimd.tensor_mul(
                                        prod2[:],
                                        cts[:, T * k + t0 : T * k + t0 + 512],
                                        wslice,
                                    )
                                    nc.vector.reduce_sum(
                                        ctx_all[:, NST * k + st : NST * k + st + 1],
                                        prod2[:],
                                        axis=mybir.AxisListType.X,
                                    )
                                elif p2_stt:
                                    # fused multiply-reduce on DVE
                                    nc.vector.scalar_tensor_tensor(
                                        out=prod2[:],
                                        in0=cts[:, T * k + t0 : T * k + t0 + 512],
                                        scalar=1.0,
                                        in1=wslice,
                                        op0=ALU.mult,
                                        op1=ALU.mult,
                                        accum_out=ctx_all[
                                            :, NST * k + st : NST * k + st + 1
                                        ],
                                    )
                                else:
                                    # DVE multiply + DVE reduce
                                    nc.vector.tensor_mul(
                                        prod2[:],
                                        cts[:, T * k + t0 : T * k + t0 + 512],
                                        wslice,
                                    )
                                    nc.vector.reduce_sum(
                                        ctx_all[:, NST * k + st : NST * k + st + 1],
                                        prod2[:],
                                        axis=mybir.AxisListType.X,
                                    )
                    if p2_wide and stage >= 6:
                        ctxs = workp.tile([128, KD], F32, tag="ctxs")
                        for k in range(KD):
                            prod2 = workp.tile([128, T], BF16, tag="prod2w")
                            nc.vector.scalar_tensor_tensor(
                                out=prod2[:],
                                in0=cts[:, T * k : T * (k + 1)],
                                scalar=1.0,
                                in1=wb_all[:],
                                op0=ALU.mult,
                                op1=ALU.mult,
                                accum_out=ctxs[:, k : k + 1],
                            )
                    if stage < 7:
                        continue
                    # ---- per-batch tail (DVE only; transpose batched at end) ----
                    stot = workp.tile([128, 1], F32, tag="stot")
                    nc.vector.reduce_sum(stot[:], s_all[:], axis=mybir.AxisListType.X)
                    invc = workp.tile([128, 1], F32, tag="invc")
                    nc.vector.reciprocal(invc[:], stot[:])
                    if not p2_wide:
                        ctxs = workp.tile([128, KD], F32, tag="ctxs")
                        for k in range(KD):
                            nc.vector.reduce_sum(
                                ctxs[:, k : k + 1],
                                ctx_all[:, NST * k : NST * (k + 1)],
                                axis=mybir.AxisListType.X,
                            )
                    nc.vector.tensor_scalar_mul(
                        ctxn_all[:, KD * b : KD * (b + 1)], ctxs[:], invc[:, 0:1]
                    )
                  if stage >= 7:
                    # ---- end-of-repeat tail: one transpose, one copy, one DMA ----
                    pst = ptp.tile([n_batch * KD, 128], F32, tag="pst")
                    nc.tensor.transpose(pst[:], ctxn_all[:], eye_f[:, :])
                    orows = workp.tile([n_batch * KD, 128], F32, tag="orows")
                    nc.scalar.activation(orows[:], pst[:], AF.Copy)
                    nc.gpsimd.dma_start(
                        out_ext.rearrange("b (k f) -> (b k) f", k=KD), orows[:]
                    )
    nc.compile()
    return nc


_NC_CACHE = None


def _get_nc():
    global _NC_CACHE
    if _NC_CACHE is None:
        _NC_CACHE = build_nc()
    return _NC_CACHE


def make_in_maps(c, h, W1_w, W1_b, W2_w, W2_b, V_w):
    c = np.asarray(c, np.float32)
    cb = c.astype(ml_dtypes.bfloat16)                    # [B, T, D] bf16
    ct = np.ascontiguousarray(cb.swapaxes(1, 2))         # [B, D, T] bf16
    shared = {
        "W1_w": np.ascontiguousarray(
            np.asarray(W1_w, np.float32).astype(ml_dtypes.bfloat16)
        ),
        "W1_b": np.ascontiguousarray(np.asarray(W1_b, np.float32)),
        "W2_w": np.ascontiguousarray(
            np.asarray(W2_w, np.float32).astype(ml_dtypes.bfloat16)
        ),
        "W2_b": np.ascontiguousarray(np.asarray(W2_b, np.float32)),
        # V columns: [p, k] = V[k*128 + p]
        "V_w": np.ascontiguousarray(
            np.asarray(V_w, np.float32).reshape(KD, 128).T
        ),
        "ones": np.ones((128, 128), np.float32),
        "eye": np.eye(128, dtype=np.float32),
    }
    h = np.asarray(h, np.float32)
    in_maps = []
    for i in range(NCORES):
        m = dict(shared)
        m["ct"] = ct[i * BL : (i + 1) * BL]
        # hT[p, k*16 + b] = h[b, k*128 + p], bf16
        hc = h[i * BL : (i + 1) * BL]                       # [BL, D]
        ht = np.zeros((128, KD * 16), np.float32)
        ht[:, :] = np.concatenate(
            [
                np.pad(hc[:, k * 128 : (k + 1) * 128].T, ((0, 0), (0, 16 - BL)))
                for k in range(KD)
            ],
            axis=1,
        )
        m["hT"] = np.ascontiguousarray(ht.astype(ml_dtypes.bfloat16))
        in_maps.append(m)
    return in_maps


def kernel(**inputs):
    in_maps = make_in_maps(
        inputs["c"], inputs["h"], inputs["W1_w"], inputs["W1_b"],
        inputs["W2_w"], inputs["W2_b"], inputs["V_w"],
    )
    nc = _get_nc()
    res = bass_utils.run_bass_kernel_spmd(nc, in_maps, core_ids=list(range(NCORES)))
    out = np.concatenate([np.asarray(r["out"]) for r in res.results], axis=0)
    return out.astype(np.float32)


if __name__ == "__main__":
    rng = np.random.default_rng(0)
    ins = {
        "c": rng.standard_normal((B, T, D), dtype=np.float32),
        "h": rng.standard_normal((B, D), dtype=np.float32),
        "W1_w": rng.standard_normal((D, U), dtype=np.float32) / np.sqrt(D),
        "W1_b": np.zeros((U,), np.float32),
        "W2_w": rng.standard_normal((D, U), dtype=np.float32) / np.sqrt(D),
        "W2_b": np.zeros((U,), np.float32),
        "V_w": rng.standard_normal((U, 1), dtype=np.float32) / np.sqrt(U),
        "V_b": np.zeros((1,), np.float32),
    }
    out = kernel(**ins)
    print("out", out.shape, out.dtype, np.abs(out).mean())
